# revision 9
# baseline (speedup 1.0000x reference)
"""Trainium2 Bass kernel for a decoder layer (self-attn + cross-attn + MLP,
custom global norm), sharded over 8 NeuronCores as 4 samples x 2 seq halves.

Layout: activations kept transposed [D, S] (d on partitions); weights used
directly as matmul lhsT tiles [d_in, d_out]. Matmuls run in float32r (TF32).
Cross-attention K/V projections are hoisted over norm1's AllReduce, and the
MLP's W1 matmul runs on pre-norm z2 via linearity (pe = a*z2 + b).
"""
import sys
sys.path.insert(0, '/opt/trn_rl_repo')
import numpy as np

B, D, S, H, DH, DFF = 4, 1024, 1024, 16, 64, 4096
N_CORES = 8
NUDGE = 1e-7
NTOT = float(D * S)
RG = [[0, 1], [2, 3], [4, 5], [6, 7]]


def round_tf32(x):
    b = np.ascontiguousarray(x, dtype=np.float32).view(np.uint32)
    return ((b + 0x1000) & 0xFFFFE000).view(np.float32)


def _split_multi_waits(nc, mybir):
    """walrus codegen allows at most one sync-wait command per instruction;
    move extra waits onto same-engine NoOps inserted just before."""
    n = 0
    for f in nc.m.functions:
        for bb in f.blocks:
            new_insts = []
            for inst in bb.instructions:
                si = getattr(inst, "sync_info", None)
                eng = getattr(inst, "engine", None)
                if si is not None and si.on_wait and len(si.on_wait) > 1 \
                        and eng is not None:
                    waits = list(si.on_wait)
                    for i, w in enumerate(waits[:-1]):
                        nop = mybir.InstNoOp(
                            name=f"{inst.name}-wsplit{i}",
                            engine=eng,
                            sync_info=mybir.SyncInfo(on_wait=[w], on_update=[]),
                            bass_nofuse=True,
                        )
                        new_insts.append(nop)
                        n += 1
                    si.on_wait = [waits[-1]]
                new_insts.append(inst)
            bb.instructions[:] = new_insts
    return n


def build_program():
    import concourse.bass as bass
    import concourse.tile as tile
    from concourse import mybir

    FP32 = mybir.dt.float32
    FP32R = mybir.dt.float32r
    AF = mybir.ActivationFunctionType
    ALU = mybir.AluOpType
    AX = mybir.AxisListType

    nc = bass.Bass("TRN2", target_bir_lowering=False, debug=False,
                   num_devices=N_CORES)

    def din(name, shape, dt=FP32R):
        return nc.dram_tensor(name, shape, dt, kind="ExternalInput").ap()

    x_d = din("x", [D, S])
    xq_d = din("xq", [D, 512])
    emb_d = din("emb", [D, S])
    mask_d = din("mask", [S, 512])
    wq_s_d = din("wq_s", [D, D]); wk_s_d = din("wk_s", [D, D])
    wv_s_d = din("wv_s", [D, D]); wo_s_d = din("wo_s", [D, D])
    wq_c_d = din("wq_c", [D, D]); wk_c_d = din("wk_c", [D, D])
    wv_c_d = din("wv_c", [D, D]); wo_c_d = din("wo_c", [D, D])
    w1_d = din("w1", [D, DFF]); w2_d = din("w2", [DFF, D])
    b1_d = din("b1m", [128, 32], FP32)
    b2_d = din("b2m", [128, 8], FP32)
    w1s_d = din("w1s", [128, 32], FP32)     # colsums of W1
    ones64_d = din("ones64", [128, 64])
    ones2_d = din("ones2", [128, 2])
    out_d = nc.dram_tensor("out", [D, 512], FP32, kind="ExternalOutput").ap()

    def r3(ap, inner):
        return ap.rearrange("(t p) m -> p t m", p=128)

    with tile.TileContext(nc) as tc:
        import contextlib
        ctx = contextlib.ExitStack()
        with ctx:
            persist = ctx.enter_context(tc.tile_pool(name="persist", bufs=1))
            dram = ctx.enter_context(
                tc.tile_pool(name="dram", bufs=1, space="DRAM"))
            actp = ctx.enter_context(tc.tile_pool(name="actp", bufs=3))

            def act_tile(nm):
                return actp.tile([128, 8, 512], FP32R, tag="act", name=nm)

            xq_sb = persist.tile([128, 8, 512], FP32R)
            nc.sync.dma_start(out=xq_sb, in_=r3(xq_d, 512))
            ones64_sb = persist.tile([128, 64], FP32R)
            nc.sync.dma_start(out=ones64_sb, in_=ones64_d)
            ones2_sb = persist.tile([128, 2], FP32R)
            nc.sync.dma_start(out=ones2_sb, in_=ones2_d)
            b1_sb = persist.tile([128, 32], FP32)
            nc.sync.dma_start(out=b1_sb, in_=b1_d)
            b2_sb = persist.tile([128, 8], FP32)
            nc.sync.dma_start(out=b2_sb, in_=b2_d)
            w1s_sb = persist.tile([128, 32], FP32)
            nc.sync.dma_start(out=w1s_sb, in_=w1s_d)

            cc_in = [dram.tile([1, 2], FP32, name=f"cc_in{i}", tag=f"cci{i}")
                     for i in range(3)]
            cc_out = [dram.tile([1, 2], FP32, name=f"cc_out{i}", tag=f"cco{i}")
                      for i in range(3)]

            def norm_stats(z_sb, cc_idx, statp):
                """Local sums -> pairwise AllReduce -> rcol/nbias columns in
                statp's st tile. Returns (rcol, nbias) APs."""
                st = statp.tile([128, 8], FP32, tag="st", name=f"st{cc_idx}")
                sqp = tc.alloc_tile_pool(name=f"sq{cc_idx}", bufs=2)
                with tc.tile_pool(name=f"npsum{cc_idx}", bufs=1,
                                  space="PSUM") as npsum:
                    n1 = npsum.tile([2, 512], FP32, tag="n1")
                    n2 = npsum.tile([2, 512], FP32, tag="n2")
                    for di in range(8):
                        sq = sqp.tile([128, 512], FP32R, tag="sq")
                        nc.vector.tensor_mul(sq, z_sb[:, di, :],
                                             z_sb[:, di, :])
                        nc.tensor.matmul(n1, ones2_sb, z_sb[:, di, :],
                                         start=(di == 0), stop=(di == 7))
                        nc.tensor.matmul(n2, ones2_sb, sq,
                                         start=(di == 0), stop=(di == 7))
                    nc.vector.tensor_reduce(st[0:1, 6:7], n1[0:1, :],
                                            AX.X, ALU.add)
                    nc.vector.tensor_reduce(st[0:1, 7:8], n2[0:1, :],
                                            AX.X, ALU.add)
                sqp.release()
                nc.sync.dma_start(out=cc_in[cc_idx], in_=st[0:1, 6:8])
                nc.gpsimd.collective_compute(
                    "AllReduce", ALU.add, replica_groups=RG,
                    ins=[cc_in[cc_idx]], outs=[cc_out[cc_idx]])
                gs = st[:, 4:6]
                bco = cc_out[cc_idx]
                bcast = bass.AP(tensor=bco.tensor, offset=bco.offset,
                                ap=[[0, 128], [1, 2]])
                nc.sync.dma_start(out=gs, in_=bcast)
                s1, s2 = gs[:, 0:1], gs[:, 1:2]
                mean, tmp = st[:, 0:1], st[:, 1:2]
                rcol, nbias = st[:, 2:3], st[:, 3:4]
                nc.vector.tensor_scalar_mul(mean, s1, 1.0 / NTOT)
                nc.vector.tensor_mul(tmp, mean, s1)
                nc.vector.tensor_sub(tmp, s2, tmp)
                nc.scalar.sqrt(tmp, tmp)
                nc.vector.tensor_scalar_add(tmp, tmp, NUDGE)
                nc.vector.reciprocal(rcol, tmp)
                nc.vector.tensor_mul(nbias, mean, rcol)
                nc.vector.tensor_scalar_mul(nbias, nbias, -1.0)
                return rcol, nbias

            def norm_apply(z_sb, dst_sb, rcol, nbias):
                for di in range(8):
                    nc.vector.tensor_scalar(dst_sb[:, di, :], z_sb[:, di, :],
                                            rcol, nbias, ALU.mult, ALU.add)

            def kproj(dst_sb, src_sb, w_dram, wpool, ppool, nsblk):
                for do in range(8):
                    wblk = wpool.tile([128, 8, 128], FP32R, tag="kw")
                    nc.sync.dma_start(
                        out=wblk,
                        in_=r3(w_dram[:, do * 128:(do + 1) * 128], 128))
                    for sb_ in range(nsblk):
                        ps = ppool.tile([128, 512], FP32, tag="pp")
                        for di in range(8):
                            nc.tensor.matmul(
                                ps, wblk[:, di, :],
                                src_sb[:, di, sb_ * 512:(sb_ + 1) * 512],
                                start=(di == 0), stop=(di == 7))
                        nc.scalar.copy(
                            dst_sb[:, do, sb_ * 512:(sb_ + 1) * 512], ps)

            def kv_projections(kv_loader, wk_dr, wv_dr, kvp, aph):
                """K^T ([d,s]) and V ([s,d]) from the kv source."""
                kt_sb = kvp.tile([128, 8, 1024], FP32R, tag="kt")
                v_sb = kvp.tile([128, 8, 1024], FP32R, tag="v")
                with (
                    tc.tile_pool(name=f"src{aph}", bufs=1) as srcp,
                    tc.tile_pool(name=f"wv{aph}", bufs=1) as wvp,
                    tc.tile_pool(name=f"wstr{aph}", bufs=2) as wpool,
                    tc.tile_pool(name=f"pp{aph}", bufs=4,
                                 space="PSUM") as ppool,
                ):
                    src_sb = kv_loader(srcp)
                    kproj(kt_sb, src_sb, wk_dr, wpool, ppool, 2)
                    for dvb in range(2):
                        wvh = wvp.tile([128, 8, 512], FP32R, tag="wv")
                        nc.sync.dma_start(
                            out=wvh,
                            in_=r3(wv_dr[:, dvb * 512:(dvb + 1) * 512], 512))
                        for st_ in range(8):
                            ps = ppool.tile([128, 512], FP32, tag="pp")
                            for di in range(8):
                                nc.tensor.matmul(
                                    ps,
                                    src_sb[:, di, st_ * 128:(st_ + 1) * 128],
                                    wvh[:, di, :],
                                    start=(di == 0), stop=(di == 7))
                            nc.vector.tensor_copy(
                                v_sb[:, st_, dvb * 512:(dvb + 1) * 512], ps)
                return kt_sb, v_sb

            def attn_rest(kt_sb, v_sb, q_src_sb, wq_dr, wo_dr, use_mask,
                          resid_sb, z_sb, aout_sb, kvp, aph):
                """Q proj, per-head attention, Wo, residual."""
                qt_sb = kvp.tile([128, 8, 512], FP32R, tag="qt")
                with (
                    tc.tile_pool(name=f"wq{aph}", bufs=2) as wpool,
                    tc.tile_pool(name=f"qp{aph}", bufs=4,
                                 space="PSUM") as ppool,
                ):
                    kproj(qt_sb, q_src_sb, wq_dr, wpool, ppool, 1)

                with (
                    tc.tile_pool(name=f"mk{aph}", bufs=1) as mkp,
                    tc.tile_pool(name=f"ep{aph}", bufs=5) as epool,
                    tc.tile_pool(name=f"dv{aph}", bufs=3) as dvp,
                    tc.tile_pool(name=f"scp{aph}", bufs=2,
                                 space="PSUM") as scp,
                    tc.tile_pool(name=f"avp{aph}", bufs=2,
                                 space="PSUM") as avp,
                ):
                    mask_sb = None
                    if use_mask:
                        mask_sb = mkp.tile([128, 8, 512], FP32R, tag="mask")
                        nc.sync.dma_start(out=mask_sb, in_=r3(mask_d, 512))
                    for h in range(H):
                        off = (h % 2) * 64
                        hp = h // 2
                        e_tiles = []
                        for tt in range(4):
                            sc = scp.tile([128, 2, 512], FP32, tag="sc")
                            for j in range(2):
                                kt = 2 * tt + j
                                nc.tensor.matmul(
                                    sc[:, j, :],
                                    kt_sb[off:off + 64, hp,
                                          kt * 128:(kt + 1) * 128],
                                    qt_sb[off:off + 64, hp, :],
                                    start=True, stop=True,
                                    tile_position=(off, 0))
                            e = epool.tile([128, 2, 512], FP32R, tag="e")
                            nc.scalar.activation(e, sc, AF.Exp, scale=0.125)
                            if mask_sb is not None:
                                nc.vector.tensor_mul(
                                    e, e, mask_sb[:, 2 * tt:2 * tt + 2, :])
                            e_tiles.append(e)
                        av = avp.tile([128, 512], FP32, tag="av")
                        dn = avp.tile([128, 512], FP32, tag="dn")
                        for kt in range(8):
                            rhs = e_tiles[kt // 2][:, kt % 2, :]
                            nc.tensor.matmul(
                                av[0:64, :],
                                v_sb[:, kt, h * 64:(h + 1) * 64], rhs,
                                start=(kt == 0), stop=(kt == 7))
                            nc.tensor.matmul(
                                dn[0:64, :], ones64_sb, rhs,
                                start=(kt == 0), stop=(kt == 7))
                        rec = dvp.tile([128, 512], FP32, tag="rec")
                        nc.vector.reciprocal(rec[0:64, :], dn[0:64, :])
                        if off == 0:
                            nc.vector.tensor_mul(aout_sb[0:64, hp, :],
                                                 av[0:64, :], rec[0:64, :])
                        else:
                            tmp = dvp.tile([128, 512], FP32R, tag="tmp")
                            nc.vector.tensor_mul(tmp[0:64, :], av[0:64, :],
                                                 rec[0:64, :])
                            nc.sync.dma_start(out=aout_sb[64:128, hp, :],
                                              in_=tmp[0:64, :])

                with (
                    tc.tile_pool(name=f"wo{aph}", bufs=2) as wpool2,
                    tc.tile_pool(name=f"wops{aph}", bufs=3,
                                 space="PSUM") as wops,
                ):
                    for do in range(8):
                        wblk = wpool2.tile([128, 8, 128], FP32R, tag="kw")
                        nc.sync.dma_start(
                            out=wblk,
                            in_=r3(wo_dr[:, do * 128:(do + 1) * 128], 128))
                        ps = wops.tile([128, 512], FP32, tag="wo")
                        for di in range(8):
                            nc.tensor.matmul(ps, wblk[:, di, :],
                                             aout_sb[:, di, :],
                                             start=(di == 0), stop=(di == 7))
                        nc.vector.tensor_add(z_sb[:, do, :], ps,
                                             resid_sb[:, do, :])

            # ================= self attention =================
            z1_sb = act_tile("z1")
            stat1 = tc.alloc_tile_pool(name="stat1", bufs=1)
            with tc.tile_pool(name="kvS", bufs=1) as kvS:

                def load_x(pool):
                    x_sb = pool.tile([128, 8, 1024], FP32R, tag="src")
                    nc.sync.dma_start(out=x_sb, in_=r3(x_d, 1024))
                    return x_sb

                ktS, vS = kv_projections(load_x, wk_s_d, wv_s_d, kvS, "s")
                aoutS = act_tile("aoutS")
                attn_rest(ktS, vS, xq_sb, wq_s_d, wo_s_d, True,
                          xq_sb, z1_sb, aoutS, kvS, "s")
            # norm1 stats: the AllReduce overlaps cross K/V projections
            rcol1, nbias1 = norm_stats(z1_sb, 0, stat1)

            # ============= cross attention =============
            stat2 = tc.alloc_tile_pool(name="stat2", bufs=1)
            with tc.tile_pool(name="kvC", bufs=1) as kvC:

                def load_emb(pool):
                    e_sb = pool.tile([128, 8, 1024], FP32R, tag="src")
                    nc.sync.dma_start(out=e_sb, in_=r3(emb_d, 1024))
                    return e_sb

                ktC, vC = kv_projections(load_emb, wk_c_d, wv_c_d, kvC, "c")
                pa_sb = act_tile("pa")
                norm_apply(z1_sb, pa_sb, rcol1, nbias1)
                aoutC = act_tile("aoutC")
                z2_sb = act_tile("z2")
                attn_rest(ktC, vC, pa_sb, wq_c_d, wo_c_d, False,
                          pa_sb, z2_sb, aoutC, kvC, "c")
                # norm2 stats start here; W1 @ z2 overlaps the AllReduce
                rcol2, nbias2 = norm_stats(z2_sb, 1, stat2)

            # ================= MLP =================
            with (
                tc.tile_pool(name="mlp", bufs=1) as mlp,
                tc.tile_pool(name="w1str", bufs=3) as w1str,
                tc.tile_pool(name="w2str", bufs=2) as w2str,
            ):
                # M = W1.T @ z2 (pre-norm); then h1 = relu(a*M + b*w1s + b1)
                m_sb = mlp.tile([128, 32, 512], FP32R, tag="h1")
                with tc.tile_pool(name="m1ps", bufs=4, space="PSUM") as m1ps:
                    for f in range(32):
                        wblk = w1str.tile([128, 8, 128], FP32R, tag="w1")
                        nc.sync.dma_start(
                            out=wblk,
                            in_=r3(w1_d[:, f * 128:(f + 1) * 128], 128))
                        ps = m1ps.tile([128, 512], FP32, tag="m1")
                        for di in range(8):
                            nc.tensor.matmul(ps, wblk[:, di, :],
                                             z2_sb[:, di, :],
                                             start=(di == 0), stop=(di == 7))
                        nc.vector.tensor_copy(m_sb[:, f, :], ps)
                # per-f bias: b*w1s + b1, then in-place relu(a*M + bias)
                biasf = mlp.tile([128, 32], FP32, tag="biasf")
                nc.vector.tensor_scalar(biasf, w1s_sb, nbias2, None, ALU.mult)
                nc.vector.tensor_add(biasf, biasf, b1_sb)
                pe_sb = act_tile("pe")
                norm_apply(z2_sb, pe_sb, rcol2, nbias2)
                h1_sb = m_sb
                for f in range(32):
                    nc.scalar.activation(h1_sb[:, f, :],
                                         m_sb[:, f, :].bitcast(FP32),
                                         AF.Relu, bias=biasf[:, f:f + 1],
                                         scale=rcol2)
                z3_sb = act_tile("z3")
                with tc.tile_pool(name="m2ps", bufs=3, space="PSUM") as m2ps:
                    for do in range(8):
                        w2blk = w2str.tile([128, 32, 128], FP32R, tag="w2")
                        nc.sync.dma_start(
                            out=w2blk,
                            in_=r3(w2_d[:, do * 128:(do + 1) * 128], 128))
                        ps = m2ps.tile([128, 512], FP32, tag="m2")
                        for ff in range(32):
                            nc.tensor.matmul(ps, w2blk[:, ff, :],
                                             h1_sb[:, ff, :],
                                             start=(ff == 0), stop=(ff == 31))
                        nc.vector.scalar_tensor_tensor(
                            z3_sb[:, do, :], ps, b2_sb[:, do:do + 1],
                            pe_sb[:, do, :], ALU.add, ALU.add)
                stat3 = tc.alloc_tile_pool(name="stat3", bufs=1)
                rcol3, nbias3 = norm_stats(z3_sb, 2, stat3)
                out_sb = mlp.tile([128, 8, 512], FP32, tag="h1")
                norm_apply(z3_sb, out_sb, rcol3, nbias3)
                nc.sync.dma_start(out=r3(out_d, 512), in_=out_sb)
                stat3.release()
            stat2.release()
            stat1.release()

    from concourse import mybir as _mb
    _split_multi_waits(nc, _mb)
    return nc


_CACHE = {}


def _get_program():
    if "nc" not in _CACHE:
        _CACHE["nc"] = build_program()
    return _CACHE["nc"]


def _make_in_maps(inputs):
    w_shared = {}
    for k in ("Wq_s", "Wk_s", "Wv_s", "Wo_s", "Wq_c", "Wk_c", "Wv_c", "Wo_c",
              "W1", "W2"):
        w_shared[k.lower()] = round_tf32(inputs[k])
    b1m = np.ascontiguousarray(
        np.asarray(inputs["b1"], np.float32).reshape(32, 128).T)
    b2m = np.ascontiguousarray(
        np.asarray(inputs["b2"], np.float32).reshape(8, 128).T)
    w1s = np.ascontiguousarray(
        w_shared["w1"].sum(axis=0, dtype=np.float64).astype(
            np.float32).reshape(32, 128).T)
    ones64 = np.ones((128, 64), np.float32)
    ones2 = np.ones((128, 2), np.float32)

    in_maps = []
    for c in range(N_CORES):
        b, h = c // 2, c % 2
        x_r = round_tf32(inputs["other_inputs"][b])
        emb_r = round_tf32(inputs["embedding"][b])
        qg = h * 512 + np.arange(512)
        mask = (np.arange(S)[:, None] <= qg[None, :]).astype(np.float32)
        m = {
            "x": x_r,
            "xq": np.ascontiguousarray(x_r[:, h * 512:(h + 1) * 512]),
            "emb": emb_r,
            "mask": mask,
            "b1m": b1m, "b2m": b2m, "w1s": w1s,
            "ones64": ones64, "ones2": ones2,
        }
        m.update(w_shared)
        in_maps.append(m)
    return in_maps


def run(inputs, trace=False):
    from concourse.bass_utils import run_bass_kernel_spmd
    nc = _get_program()
    in_maps = _make_in_maps(inputs)
    res = run_bass_kernel_spmd(nc, in_maps, list(range(N_CORES)), trace=trace)
    out = np.zeros((B, D, S), np.float32)
    for c in range(N_CORES):
        b, h = c // 2, c % 2
        out[b][:, h * 512:(h + 1) * 512] = res.results[c]["out"]
    return out, res


def kernel(**inputs):
    out, _ = run(inputs, trace=False)
    return out
